# revision 1
# baseline (speedup 1.0000x reference)
"""Trainium2 Bass kernel: 4-layer MLP (784-512-512-512-10) + log_softmax.

Data-parallel over 8 NeuronCores: batch 65536 is split into 8 shards of
8192 rows; the ~1M-param weights are replicated on every core.

Layout: activations live on-chip transposed ([features, batch]) so every
layer's matmul is `out[of, nb] += W_lT[if, of].T @ h[if, nb]` with the
feature chunks on partitions.  Layers 1-3 run in fp8e4 with DoubleRow
(two 128-feature contraction chunks per matmul, fp32 PSUM accumulate);
layer 4 runs in bf16 with the batch flipped onto PSUM partitions so the
softmax reduces along the free dim.  Batch is processed in superchunks
of 1024 rows: each output-feature chunk accumulates two 512-column
halves into one 2-bank PSUM tile so a single 1024-wide op applies
bias+ReLU (all on ScalarE, the faster PSUM drainer — measured best).  log_softmax skips the max-subtraction (logits are small
enough for fp32 exp), accumulates sum(exp) via the Exp activation's
accum_out, and applies a batched Ln + broadcast-subtract epilogue, split
so most of it hides under the last superchunk's matmuls.  A short dummy-
matmul warm-up keeps the PE HAM clock gate at 2.4 GHz through the
initial DMA wait.

Measured on 8 axon trn2 cores: ~169-171 us HW exec per NEFF
(~212 us when the device sits in its throttled power state),
absmax 1.39e-2 / scale-relative 5.6e-3 / max-relative 6.1e-3 vs the
fp32 jax reference (fp8 quantization dominates the error; the all-bf16
variant measured 257 us at 4.3e-4 max-relative).
"""

from contextlib import ExitStack

import ml_dtypes
import numpy as np

import concourse.bass as bass  # noqa: F401  (registers AP machinery)
from concourse import bacc, mybir
from concourse.bass_utils import run_bass_kernel_spmd
from concourse.tile import TileContext

BF16 = mybir.dt.bfloat16
FP32 = mybir.dt.float32
FP8 = mybir.dt.float8e4

N_CORES = 8
B = 65536
D0, H, C = 784, 512, 10
BC = B // N_CORES            # 8192 rows per core
NB = 512                     # matmul moving free dim / PSUM bank width
HB = 2                       # batch halves sharing one PSUM group
SNB = NB * HB                # 1024-row superchunk
NCHUNK = BC // SNB           # 8 superchunks
K0F = D0 // 128              # 6 full 128-row contraction chunks in layer 1
K0R = D0 - K0F * 128         # 16 remainder rows
KH = H // 128                # 4 contraction chunks for hidden layers
NRG = BC // 128              # 64 row-groups of 128 rows per core

_CACHED_NC = None


def build_nc():
    nc = bacc.Bacc(
        "TRN2",
        target_bir_lowering=False,
        debug=False,
        enable_asserts=False,
        num_devices=N_CORES,
    )
    xt_d = nc.declare_dram_parameter("xt", [D0, BC], FP8, isOutput=False)
    w1_d = nc.declare_dram_parameter("w1t", [D0, H], FP8, isOutput=False)
    w2_d = nc.declare_dram_parameter("w2t", [H, H], FP8, isOutput=False)
    w3_d = nc.declare_dram_parameter("w3t", [H, H], FP8, isOutput=False)
    w4_d = nc.declare_dram_parameter("w4t", [H, C], BF16, isOutput=False)
    b1_d = nc.declare_dram_parameter("b1", [H], FP32, isOutput=False)
    b2_d = nc.declare_dram_parameter("b2", [H], FP32, isOutput=False)
    b3_d = nc.declare_dram_parameter("b3", [H], FP32, isOutput=False)
    b4_d = nc.declare_dram_parameter("b4r", [128, C], FP32, isOutput=False)
    out_d = nc.declare_dram_parameter("out", [BC, C], FP32, isOutput=True)

    expf = mybir.ActivationFunctionType.Exp
    reluf = mybir.ActivationFunctionType.Relu
    lnf = mybir.ActivationFunctionType.Ln
    add_op = mybir.AluOpType.add
    max_op = mybir.AluOpType.max
    sub_op = mybir.AluOpType.subtract
    drow = mybir.MatmulPerfMode.DoubleRow

    with TileContext(nc) as tc, ExitStack() as ctx:
        consts = ctx.enter_context(tc.tile_pool(name="consts", bufs=1))
        xpool = ctx.enter_context(tc.tile_pool(name="xp", bufs=6))
        hpool = ctx.enter_context(tc.tile_pool(name="hp", bufs=3))
        spool = ctx.enter_context(tc.tile_pool(name="sp", bufs=4))
        pbig = ctx.enter_context(tc.tile_pool(name="pbig", bufs=3, space="PSUM"))
        psml = ctx.enter_context(tc.tile_pool(name="psml", bufs=2, space="PSUM"))

        # Resident weights/biases, loaded once (ScalarE DMA queue so the
        # SP queue is free for the first x superchunk).
        w1 = consts.tile([128, K0F + 1, H], FP8, tag="w1")
        for k in range(K0F):
            nc.scalar.dma_start(w1[:, k, :], w1_d[k * 128 : (k + 1) * 128, :])
        nc.scalar.dma_start(w1[0:K0R, K0F, :], w1_d[K0F * 128 : D0, :])
        w2 = consts.tile([128, KH, H], FP8, tag="w2")
        nc.scalar.dma_start(w2[:], w2_d.rearrange("(o p) n -> p o n", p=128))
        w3 = consts.tile([128, KH, H], FP8, tag="w3")
        nc.scalar.dma_start(w3[:], w3_d.rearrange("(o p) n -> p o n", p=128))
        w4 = consts.tile([128, KH, C], BF16, tag="w4")
        nc.scalar.dma_start(w4[:], w4_d.rearrange("(o p) n -> p o n", p=128))
        b1s = consts.tile([128, KH], FP32, tag="b1")
        nc.scalar.dma_start(b1s[:], b1_d.rearrange("(o p) -> p o", p=128))
        b2s = consts.tile([128, KH], FP32, tag="b2")
        nc.scalar.dma_start(b2s[:], b2_d.rearrange("(o p) -> p o", p=128))
        b3s = consts.tile([128, KH], FP32, tag="b3")
        nc.scalar.dma_start(b3s[:], b3_d.rearrange("(o p) -> p o", p=128))
        b4s = consts.tile([128, C], FP32, tag="b4")
        nc.scalar.dma_start(b4s[:], b4_d[:])

        # PE warm-up: ~3.5us of dummy matmuls during the initial DMA wait
        # so the HAM clock gate is at 2.4 GHz when real work arrives.
        warm = consts.tile([128, NB], FP8, tag="warm")
        nc.vector.memset(warm[:], 1.0)
        psw = pbig.tile([128, HB, NB], FP32, tag="ps", name="ps_warm")
        for i in range(26):
            nc.tensor.matmul(
                psw[:, i % 2, :], lhsT=warm[:, 0:128], rhs=warm[:],
                start=(i < 2), stop=(i >= 24),
            )

        # Persistent softmax state for all 64 row-groups.
        logits_all = consts.tile([128, NRG, C], FP32, tag="logits_all")
        esum_all = consts.tile([128, NRG], FP32, tag="esum_all")
        lns_all = consts.tile([128, NRG], FP32, tag="lns_all")
        obuf = consts.tile([128, NRG, C], FP32, tag="obuf")

        def softmax_epilogue(rg0, rg1):
            # out = logits - ln(sum(exp(logits))) for row-groups [rg0, rg1)
            n = rg1 - rg0
            nc.scalar.activation(lns_all[:, rg0:rg1], esum_all[:, rg0:rg1], lnf)
            nc.vector.tensor_tensor(
                obuf[:, rg0:rg1, :], logits_all[:, rg0:rg1, :],
                lns_all[:, rg0:rg1, None].to_broadcast((128, n, C)), sub_op,
            )
            nc.sync.dma_start(
                out_d[rg0 * 128 : rg1 * 128, :].rearrange("(o p) n -> p o n", p=128),
                obuf[:, rg0:rg1, :],
            )

        for sc in range(NCHUNK):
            b0 = sc * SNB
            xt = xpool.tile([128, K0F + 1, SNB], FP8, tag="xt")
            for k in range(K0F):
                nc.sync.dma_start(
                    xt[:, k, :], xt_d[k * 128 : (k + 1) * 128, b0 : b0 + SNB]
                )
            nc.sync.dma_start(xt[0:K0R, K0F, :], xt_d[K0F * 128 : D0, b0 : b0 + SNB])

            # Layer 1 [784 -> 512]: fp8 DoubleRow, K=16 remainder plain fp8.
            # Both batch halves accumulate into one 2-bank PSUM tile so a
            # single DVE op applies bias+ReLU to the full superchunk row.
            h1p = [
                hpool.tile([128, 2, HB, NB], FP8, tag=f"h1p_{j}", name=f"h1p_{j}")
                for j in range(KH // 2)
            ]
            for m in range(KH):
                ps = pbig.tile([128, HB, NB], FP32, tag="ps")
                ms = slice(m * 128, (m + 1) * 128)
                # DoubleRow pairs first, K=16 remainder last: the
                # group opens with a plain DR matmul (no mode-switch stall)
                # and superchunk 0's first matmuls only need k-chunk 0/1.
                for k in range(0, K0F, 2):
                    for hb in range(HB):
                        nc.tensor.matmul(
                            ps[:, hb, :], lhsT=w1[:, k : k + 2, ms],
                            rhs=xt[:, k : k + 2, hb * NB : (hb + 1) * NB],
                            start=(k == 0), stop=False, perf_mode=drow,
                        )
                for hb in range(HB):
                    nc.tensor.matmul(
                        ps[:, hb, :], lhsT=w1[0:K0R, K0F, ms],
                        rhs=xt[0:K0R, K0F, hb * NB : (hb + 1) * NB],
                        start=False, stop=True, perf_mode=None,
                    )
                nc.scalar.activation(
                    h1p[m // 2][:, m % 2, :, :], ps[:], reluf,
                    bias=b1s[:, m : m + 1],
                )

            # Layer 2 [512 -> 512]: fp8 DoubleRow over feature-chunk pairs.
            h2p = [
                hpool.tile([128, 2, HB, NB], FP8, tag=f"h2p_{j}", name=f"h2p_{j}")
                for j in range(KH // 2)
            ]
            for m in range(KH):
                ps = pbig.tile([128, HB, NB], FP32, tag="ps")
                ms = slice(m * 128, (m + 1) * 128)
                for j in range(KH // 2):
                    for hb in range(HB):
                        nc.tensor.matmul(
                            ps[:, hb, :], lhsT=w2[:, 2 * j : 2 * j + 2, ms],
                            rhs=h1p[j][:, :, hb, :],
                            start=(j == 0), stop=(j == KH // 2 - 1),
                            perf_mode=drow,
                        )
                nc.scalar.activation(
                    h2p[m // 2][:, m % 2, :, :], ps[:], reluf,
                    bias=b2s[:, m : m + 1],
                )

            # Layer 3 [512 -> 512]: fp8 DoubleRow in, bf16 out (layer-4 lhsT).
            h3 = [
                hpool.tile([128, HB, NB], BF16, tag=f"h3_{m}", name=f"h3_{m}")
                for m in range(KH)
            ]
            for m in range(KH):
                ps = pbig.tile([128, HB, NB], FP32, tag="ps")
                ms = slice(m * 128, (m + 1) * 128)
                for j in range(KH // 2):
                    for hb in range(HB):
                        nc.tensor.matmul(
                            ps[:, hb, :], lhsT=w3[:, 2 * j : 2 * j + 2, ms],
                            rhs=h2p[j][:, :, hb, :],
                            start=(j == 0), stop=(j == KH // 2 - 1),
                            perf_mode=drow,
                        )
                nc.scalar.activation(h3[m][:], ps[:], reluf, bias=b3s[:, m : m + 1])

            # Layer 4 [512 -> 10], bf16, output flipped to [batch, 10].
            # All 8 row-groups of the superchunk accumulate into one PSUM
            # bank, so bias-add / exp / sum(exp) run as 3 batched ops.
            MG = SNB // 128
            rg0 = sc * MG
            ps4 = psml.tile([128, MG, C], FP32, tag="ps4")
            for hb in range(HB):
                for mm in range(NB // 128):
                    r = hb * (NB // 128) + mm
                    ms = slice(mm * 128, (mm + 1) * 128)
                    for k in range(KH):
                        nc.tensor.matmul(
                            ps4[:, r, :], lhsT=h3[k][:, hb, ms], rhs=w4[:, k, :],
                            start=(k == 0), stop=(k == KH - 1),
                        )
            lg = logits_all[:, rg0 : rg0 + MG, :]
            nc.vector.tensor_tensor(
                lg, ps4[:], b4s[:, None, :].to_broadcast((128, MG, C)), add_op
            )
            etile = spool.tile([128, MG, C], FP32, tag="etile")
            nc.scalar.activation(etile[:], lg, expf)
            nc.vector.tensor_reduce(
                esum_all[:, rg0 : rg0 + MG], etile[:],
                axis=mybir.AxisListType.X, op=add_op,
            )
            if sc == NCHUNK - 2:
                # Most of the epilogue hides under the last superchunk.
                softmax_epilogue(0, (NCHUNK - 1) * (SNB // 128))

        softmax_epilogue((NCHUNK - 1) * (SNB // 128), NRG)

    nc.compile()
    return nc


def _get_nc():
    global _CACHED_NC
    if _CACHED_NC is None:
        _CACHED_NC = build_nc()
    return _CACHED_NC


def make_in_maps(x, W1, b1, W2, b2, W3, b3, W4, b4):
    bf16 = ml_dtypes.bfloat16
    fp8 = ml_dtypes.float8_e4m3
    xq = np.asarray(x).astype(fp8)
    common = {
        "w1t": np.ascontiguousarray(np.asarray(W1).T.astype(fp8)),
        "w2t": np.ascontiguousarray(np.asarray(W2).T.astype(fp8)),
        "w3t": np.ascontiguousarray(np.asarray(W3).T.astype(fp8)),
        "w4t": np.ascontiguousarray(np.asarray(W4).T.astype(bf16)),
        "b1": np.asarray(b1).astype(np.float32),
        "b2": np.asarray(b2).astype(np.float32),
        "b3": np.asarray(b3).astype(np.float32),
        "b4r": np.tile(np.asarray(b4).astype(np.float32)[None, :], (128, 1)),
    }
    in_maps = []
    for i in range(N_CORES):
        shard = np.ascontiguousarray(xq[i * BC : (i + 1) * BC].T)  # [784, 8192]
        in_maps.append({"xt": shard, **common})
    return in_maps


def kernel(x, W1, b1, W2, b2, W3, b3, W4, b4):
    in_maps = make_in_maps(x, W1, b1, W2, b2, W3, b3, W4, b4)
    nc = _get_nc()
    res = run_bass_kernel_spmd(nc, in_maps, list(range(N_CORES)))
    out = np.concatenate(
        [res.results[i]["out"] for i in range(N_CORES)], axis=0
    ).astype(np.float32)
    return out

